# revision 13
# baseline (speedup 1.0000x reference)
"""Multi-head self-attention (B=4, S=2048, D=1024, H=16) on 8 TRN2 NeuronCores.

Sharding: core i = (batch b = i//2, head-group g = i%2): 8 heads per core,
Megatron-style partial output projection summed on host.

v2 (bf16): all matmuls in bf16 (PE runs bf16 at 1 col/cycle with ~0
per-instruction overhead vs ~90 cycles for f32r). Scores use the PE
quadrant-tiling: head 2f lives on partitions 0-63 of qt/kt f-tiles, head
2f+1 on 64-127; the two K=64 score matmuls of a head pair execute in
disjoint quadrant rows and overlap on HW (measured 108ns each for N=512,
i.e. 2x). Softmax denominators come free from a ones-column in V_aug
(M=65 PV matmuls cost the same as M=128). exp runs on ACT (the critical
path, ~277us); emission order starts it as early as possible (K proj ->
V proj -> per-s-chunk Q proj + attention) and keeps it streaming.
"""
import os
import sys
import types

import numpy as np

# ---------------------------------------------------------------------------
# environment bootstrap (self-contained: no problem-dir imports)
# ---------------------------------------------------------------------------


def _install_ntff_hook():
    """run_bass_kernel_spmd(trace=True) under axon needs antenv.axon_hooks,
    which the agent image's antenv stub lacks. Recreate it."""
    if "antenv.axon_hooks" in sys.modules:
        return
    try:
        import antenv
        from trn_agent_boot.trn_boot import _ntff_profile_via_ctypes
    except Exception:
        return
    so_path = "/opt/axon/libaxon_pjrt.so"
    if not os.path.exists(so_path):
        return
    mod = types.ModuleType("antenv.axon_hooks")
    _hook = [_ntff_profile_via_ctypes(so_path)]
    mod.get_axon_ntff_profile_hook = lambda: _hook[0]

    def _set(h):
        _hook[0] = h

    mod.set_axon_ntff_profile_hook = _set
    sys.modules["antenv.axon_hooks"] = mod
    antenv.axon_hooks = mod


_install_ntff_hook()

import concourse.bacc as bacc
import concourse.tile as tile
from concourse import mybir
from concourse.bass_utils import run_bass_kernel_spmd
from contextlib import ExitStack

# ---------------------------------------------------------------------------
# problem constants (hardcoded per contract)
# ---------------------------------------------------------------------------
B, S, D = 4, 2048, 1024
H, HD = 16, 64
HPG = 8            # heads per core (group)
E = HPG * HD       # 512 attention features per core
P = 128
SC = 512           # s-chunk
NS = S // SC       # 4 s-chunks
NT = S // P        # 16 t-chunks
ND = D // P        # 8 d-chunks
NF = E // P        # 4 f-tiles (head pairs)
HD1 = HD + 1       # V_aug columns per head (V + ones)
SCALE = 1.0 / np.sqrt(np.float32(HD))

F32 = mybir.dt.float32
BF16 = mybir.dt.bfloat16
EXP = mybir.ActivationFunctionType.Exp

_NC_CACHE = {}


def _build_nc():
    nc = bacc.Bacc("TRN2", target_bir_lowering=False)

    xT = nc.dram_tensor("xT", [D, S], BF16, kind="ExternalInput")
    wqT = nc.dram_tensor("wqT", [D, E], BF16, kind="ExternalInput")
    wkT = nc.dram_tensor("wkT", [D, E], BF16, kind="ExternalInput")
    wvT = nc.dram_tensor("wvT", [D, E], BF16, kind="ExternalInput")
    woT = nc.dram_tensor("woT", [E, D], BF16, kind="ExternalInput")
    bq = nc.dram_tensor("bq", [E, 1], F32, kind="ExternalInput")
    bk = nc.dram_tensor("bk", [E, 1], F32, kind="ExternalInput")
    bv = nc.dram_tensor("bv", [1, E], F32, kind="ExternalInput")
    bo = nc.dram_tensor("bo", [D, 1], F32, kind="ExternalInput")
    outT = nc.dram_tensor("outT", [D, S], F32, kind="ExternalOutput")

    with tile.TileContext(nc) as tc, ExitStack() as glob:
        const = glob.enter_context(tc.tile_pool(name="const", bufs=1))
        # resident inputs
        xr = [const.tile([P, S], BF16, name=f"xr{d}") for d in range(ND)]
        wq = [const.tile([P, E], BF16, name=f"wq{d}") for d in range(ND)]
        wk = [const.tile([P, E], BF16, name=f"wk{d}") for d in range(ND)]
        wv = [const.tile([P, E], BF16, name=f"wv{d}") for d in range(ND)]
        wo = [const.tile([P, D], BF16, name=f"wo{e}") for e in range(NF)]
        # DMA order: K-proj inputs first (they gate the first scores)
        for d in range(ND):
            nc.sync.dma_start(xr[d][:], xT[d * P:(d + 1) * P, :])
            nc.sync.dma_start(wk[d][:], wkT[d * P:(d + 1) * P, :])
        for d in range(ND):
            nc.sync.dma_start(wq[d][:], wqT[d * P:(d + 1) * P, :])
        for d in range(ND):
            nc.sync.dma_start(wv[d][:], wvT[d * P:(d + 1) * P, :])
        for e in range(NF):
            nc.sync.dma_start(wo[e][:], woT[e * P:(e + 1) * P, :])
        bqt = [const.tile([P, 1], F32, name=f"bqt{f}") for f in range(NF)]
        bkt = [const.tile([P, 1], F32, name=f"bkt{f}") for f in range(NF)]
        for f in range(NF):
            nc.sync.dma_start(bqt[f][:], bq[f * P:(f + 1) * P, :])
            nc.sync.dma_start(bkt[f][:], bk[f * P:(f + 1) * P, :])
        bv_bc = const.tile([P, E], F32, name="bv_bc")
        nc.sync.dma_start(bv_bc[:], bv[0:1, :].to_broadcast((P, E)))
        bot = [const.tile([P, 1], F32, name=f"bot{i}") for i in range(ND)]
        for i in range(ND):
            nc.sync.dma_start(bot[i][:], bo[i * P:(i + 1) * P, :])

        resid = glob.enter_context(tc.tile_pool(name="resid", bufs=1))
        qt = [resid.tile([P, S], BF16, name=f"qt{f}") for f in range(NF)]
        kt = [resid.tile([P, S], BF16, name=f"kt{f}") for f in range(NF)]
        vt = [resid.tile([P, HPG * HD1], BF16, name=f"vt{t}") for t in range(NT)]

        # psum budget (8 banks): sc ring 2x[128,1024]=4, o pool 2x[65,512]=2
        # (bufs=1; eviction slack comes from the delayed PV chain start),
        # proj ring 2x[128,512]=2 for all projection/out-proj chains.
        R = glob.enter_context(tc.tile_pool(name="R", bufs=2, space="PSUM"))
        PJ = glob.enter_context(tc.tile_pool(name="PJ", bufs=2, space="PSUM"))
        opool = glob.enter_context(tc.tile_pool(name="opool", bufs=1, space="PSUM"))

        pt_pool = glob.enter_context(tc.tile_pool(name="pt", bufs=16))
        oc_pool = glob.enter_context(tc.tile_pool(name="oc", bufs=2))
        nrm_pool = glob.enter_context(tc.tile_pool(name="nrm", bufs=2))
        on_pool = glob.enter_context(tc.tile_pool(name="on", bufs=2))
        ot_pool = glob.enter_context(tc.tile_pool(name="ot", bufs=3))
        dram_pool = glob.enter_context(tc.tile_pool(name="dramrs", bufs=2, space="DRAM"))

        # ------------------------------------------------------------------
        # Emission engine: ACT (exp) is the critical path at ~1.08us per
        # [128,1024] tile; the PE must stream score tiles at that cadence
        # while folding ALL projection work into the leftover slots so it
        # never idles long (PE DVFS: stalls drop the clock to 1.2GHz).
        # ------------------------------------------------------------------

        class FillerStream:
            """Pops projection matmuls a few at a time, managing chain state."""

            def __init__(self):
                self.chains = []   # list of (mm_list, fin_fn)
                self.cur = None
                self.idx = 0

            def push_chain(self, mms, fin):
                self.chains.append((mms, fin))

            def pop(self, n):
                done = 0
                while done < n:
                    if self.cur is None:
                        if not self.chains:
                            return done
                        self.cur = self.chains.pop(0)
                        self.idx = 0
                    mms, fin = self.cur
                    mms[self.idx]()
                    self.idx += 1
                    done += 1
                    if self.idx == len(mms):
                        fin()
                        self.cur = None
                return done

        fill = FillerStream()

        def kproj_chain(f, s):
            sl = slice(s * SC, (s + 1) * SC)
            box = {}

            def mk(d):
                def mm():
                    if d == 0:
                        box["t"] = PJ.tile([P, SC], F32, name="kps", tag="PJ")
                    nc.tensor.matmul(
                        box["t"][:], wk[d][:, f * P:(f + 1) * P], xr[d][:, sl],
                        start=(d == 0), stop=(d == ND - 1))
                return mm

            def fin():
                nc.vector.tensor_scalar_add(kt[f][:, sl], box["t"][:], bkt[f][:])

            return [mk(d) for d in range(ND)], fin

        def qproj_chain(f, s):
            sl = slice(s * SC, (s + 1) * SC)
            box = {}

            def mk(d):
                def mm():
                    if d == 0:
                        box["t"] = PJ.tile([P, SC], F32, name="qps", tag="PJ")
                    nc.tensor.matmul(
                        box["t"][:], wq[d][:, f * P:(f + 1) * P], xr[d][:, sl],
                        start=(d == 0), stop=(d == ND - 1))
                return mm

            def fin():
                nc.vector.tensor_scalar_add(qt[f][:, sl], box["t"][:], bqt[f][:])

            return [mk(d) for d in range(ND)], fin

        def vproj_chain(t):
            d0 = t * P
            box = {}

            def mk(d):
                def mm():
                    if d == 0:
                        box["t"] = PJ.tile([P, SC], F32, name="vps", tag="PJ")
                    nc.tensor.matmul(
                        box["t"][:], xr[d][:, d0:d0 + P], wv[d][:],
                        start=(d == 0), stop=(d == ND - 1))
                return mm

            def fin():
                vdst = vt[t][:].rearrange("p (h c) -> p h c", c=HD1)
                nc.vector.tensor_add(
                    vdst[:, :, 0:HD],
                    box["t"][:].rearrange("p (h c) -> p h c", c=HD),
                    bv_bc[:].rearrange("p (h c) -> p h c", c=HD))
                nc.vector.memset(vdst[:, :, HD:HD1], 1.0)

            return [mk(d) for d in range(ND)], fin

        def outproj_chain(dc, s, on_tiles):
            sl = slice(s * SC, (s + 1) * SC)
            box = {}

            def mk(e):
                def mm():
                    if e == 0:
                        box["t"] = PJ.tile([P, SC], F32, name="opps", tag="PJ")
                    nc.tensor.matmul(
                        box["t"][:], wo[e][:, dc * P:(dc + 1) * P], on_tiles[e][:],
                        start=(e == 0), stop=(e == NF - 1))
                return mm

            def fin():
                ot = ot_pool.tile([P, SC], F32, name="ottile", tag="ot")
                nc.vector.tensor_scalar_add(ot[:], box["t"][:], bot[dc][:])
                nc.sync.dma_start(outT[dc * P:(dc + 1) * P, sl], ot[:])

            return [mk(e) for e in range(NF)], fin

        # score + exp for global step g
        pt_tiles = {}

        def sc_exp(g):
            s, hp, t = g // 64, (g // 16) % 4, g % 16
            sl = slice(s * SC, (s + 1) * SC)
            tsl = slice(t * P, (t + 1) * P)
            sc_ps = R.tile([P, 2 * SC], F32, name="scps", tag="R")
            nc.tensor.matmul(
                sc_ps[:, 0:SC], kt[hp][0:HD, tsl], qt[hp][0:HD, sl],
                start=True, stop=True, tile_position=(0, 0))
            nc.tensor.matmul(
                sc_ps[:, SC:2 * SC], kt[hp][HD:P, tsl], qt[hp][HD:P, sl],
                start=True, stop=True, tile_position=(HD, 0))
            pt = pt_pool.tile([P, 2 * SC], BF16, name="ptile", tag="pt")
            nc.scalar.activation(pt[:], sc_ps[:], EXP, scale=float(SCALE))
            pt_tiles[g] = pt

        # PV pair for global step g' (consumes pt_tiles[g'])
        ostate = {}   # block -> (o_psA, o_psB)
        on_all = {}   # s -> [on tiles]

        def pv_pair(gp):
            blk = gp // 16
            s, hp, t = gp // 64, (gp // 16) % 4, gp % 16
            hA, hB = 2 * hp, 2 * hp + 1
            if t == 0:
                ostate[blk] = (
                    opool.tile([HD1, SC], F32, name="opsA", tag="oA"),
                    opool.tile([HD1, SC], F32, name="opsB", tag="oB"))
            o_psA, o_psB = ostate[blk]
            pt = pt_tiles.pop(gp)
            nc.tensor.matmul(
                o_psA[:], vt[t][:, hA * HD1:(hA + 1) * HD1], pt[:, 0:SC],
                start=(t == 0), stop=(t == NT - 1))
            nc.tensor.matmul(
                o_psB[:], vt[t][:, hB * HD1:(hB + 1) * HD1], pt[:, SC:2 * SC],
                start=(t == 0), stop=(t == NT - 1))
            if t == NT - 1:
                norm_block(s, hp, o_psA, o_psB)

        def norm_block(s, hp, o_psA, o_psB):
            """Evict O_aug, normalize by the ones-row sums into on_all[s][hp]."""
            ocA = oc_pool.tile([HD1, SC], F32, name="ocA", tag="ocA")
            ocB = oc_pool.tile([HD1, SC], F32, name="ocB", tag="ocB")
            nc.vector.tensor_copy(ocA[:], o_psA[:])
            nc.vector.tensor_copy(ocB[:], o_psB[:])
            rcpf = nrm_pool.tile([2, SC], F32, name="rcpf", tag="rcpf")
            nc.sync.dma_start(rcpf[0:1, :], ocA[HD:HD1, :])
            nc.sync.dma_start(rcpf[1:2, :], ocB[HD:HD1, :])
            rcpv = nrm_pool.tile([2, SC], F32, name="rcpv", tag="rcpv")
            nc.vector.reciprocal_approx_fast(rcpv[:], rcpf[:])
            rd = dram_pool.tile([2, SC], F32, name="rdtile", tag="rd")
            nc.sync.dma_start(rd[:, :], rcpv[:])
            rbA = nrm_pool.tile([HD, SC], F32, name="rbA", tag="rbA")
            rbB = nrm_pool.tile([HD, SC], F32, name="rbB", tag="rbB")
            nc.sync.dma_start(rbA[:], rd[0:1, :].to_broadcast((HD, SC)))
            nc.sync.dma_start(rbB[:], rd[1:2, :].to_broadcast((HD, SC)))
            on = on_all[s][hp]
            nc.vector.tensor_mul(on[0:HD, :], ocA[0:HD, :], rbA[:])
            tmpB = nrm_pool.tile([HD, SC], BF16, name="tmpB", tag="tmpB")
            nc.vector.tensor_mul(tmpB[:], ocB[0:HD, :], rbB[:])
            nc.sync.dma_start(on[HD:P, :], tmpB[:])
            if hp == NF - 1:
                for dc in range(ND):
                    fill.push_chain(*outproj_chain(dc, s, on_all[s]))

        # ---------------- prefix ------------------------------------------
        on_all[0] = [on_pool.tile([P, SC], BF16, name="on", tag=f"on{hp}")
                     for hp in range(NF)]
        # K f0 (all four s-chunk chains), Q s0 f0 -- inline, back to back
        for s in range(NS):
            mms, fin = kproj_chain(0, s)
            for m in mms:
                m()
            fin()
        mms, fin = qproj_chain(0, 0)
        for m in mms:
            m()
        fin()
        # V projection interleaved with hp0's scores (feeds ACT early)
        for t in range(NT):
            sc_exp(t)
            mms, fin = vproj_chain(t)
            for m in mms:
                m()
            fin()
        # K f1 + Q s0 f1 before the steady loop needs them
        for s in range(NS):
            mms, fin = kproj_chain(1, s)
            for m in mms:
                m()
            fin()
        mms, fin = qproj_chain(1, 0)
        for m in mms:
            m()
        fin()

        # ---------------- steady loop -------------------------------------
        DELAY = 8      # PV pointer trails the sc pointer by >= DELAY steps
        GAP = 3        # steps between a chain's stop and the next chain start
        pv_ptr = 0
        last_stop = [-10]

        def chase(g, maxpairs):
            nonlocal pv_ptr
            emitted = 0
            while emitted < maxpairs and pv_ptr <= g - DELAY and pv_ptr < 256:
                if pv_ptr % 16 == 0 and g - last_stop[0] < GAP:
                    break
                pv_pair(pv_ptr)
                if pv_ptr % 16 == 15:
                    last_stop[0] = g
                pv_ptr += 1
                emitted += 1
            return emitted

        for g in range(NT, 4 * 64):
            s, hp, t = g // 64, (g // 16) % 4, g % 16
            if t == 0:
                if hp == 0:
                    on_all[s] = [on_pool.tile([P, SC], BF16, name="on",
                                              tag=f"on{q}") for q in range(NF)]
                    if 0 < s < NS - 1:
                        for f in range(NF):
                            fill.push_chain(*qproj_chain(f, s + 1))
                # s0: K/Q for the next head pair, Q chain first (tightest
                # deadline: next block's first scores read it immediately)
                if s == 0 and 1 <= hp < NF - 1:
                    fill.push_chain(*qproj_chain(hp + 1, 0))
                    for sk in range(NS):
                        fill.push_chain(*kproj_chain(hp + 1, sk))
                if s == 0 and hp == NF - 1:
                    for f in range(NF):
                        fill.push_chain(*qproj_chain(f, 1))
            # fillers FIRST: their writes must precede this step's score
            # matmuls in program order (the tile framework only orders
            # writes->reads that appear in emission order)
            fill.pop((4 if g < 40 else 3) if s == 0 else 1)
            sc_exp(g)
            npairs = chase(g, 3)
            if s > 0 and npairs < 2:
                fill.pop(2 - npairs)
        # drain: remaining PV pairs, then leftover fillers (incl. outproj s3)
        g = 4 * 64
        while pv_ptr < 256:
            chase(g, 4)
            fill.pop(2)
            g += 1
        while fill.pop(8):
            pass

    nc.finalize()
    return nc


def _get_nc():
    if "nc" not in _NC_CACHE:
        _NC_CACHE["nc"] = _build_nc()
    return _NC_CACHE["nc"]


def _shard_inputs(x, w_qkv, b_qkv, w_out, b_out):
    """Build the 8 per-core input maps. Core i = (b = i//2, g = i%2)."""
    import ml_dtypes
    bf16 = ml_dtypes.bfloat16
    x = np.asarray(x, np.float32)
    w_qkv = np.asarray(w_qkv, np.float32)
    b_qkv = np.asarray(b_qkv, np.float32)
    w_out = np.asarray(w_out, np.float32)
    b_out = np.asarray(b_out, np.float32)

    in_maps = []
    for b in range(B):
        xTb = np.ascontiguousarray(x[b].T.astype(bf16))  # [D, S]
        for g in range(2):
            heads = range(g * HPG, (g + 1) * HPG)
            # w_qkv rows for head h: [192h, 192h+64) = Q, +64..128 = K, +128..192 = V
            q_rows = np.concatenate(
                [np.arange(3 * HD * h, 3 * HD * h + HD) for h in heads])
            k_rows = q_rows + HD
            v_rows = q_rows + 2 * HD
            wqT = np.ascontiguousarray(w_qkv[q_rows].T.astype(bf16))  # [D, E]
            wkT = np.ascontiguousarray(w_qkv[k_rows].T.astype(bf16))
            wvT = np.ascontiguousarray(w_qkv[v_rows].T.astype(bf16))
            ecols = np.arange(g * E, (g + 1) * E)
            woT = np.ascontiguousarray(w_out[:, ecols].T.astype(bf16))  # [E, D]
            bo_ = b_out[:, None] if g == 0 else np.zeros((D, 1), np.float32)
            in_maps.append({
                "xT": xTb,
                "wqT": wqT,
                "wkT": wkT,
                "wvT": wvT,
                "woT": woT,
                "bq": np.ascontiguousarray(b_qkv[q_rows][:, None]),
                "bk": np.ascontiguousarray(b_qkv[k_rows][:, None]),
                "bv": np.ascontiguousarray(b_qkv[v_rows][None, :]),
                "bo": np.ascontiguousarray(bo_),
            })
    return in_maps


def run(inputs, trace=False):
    """Run the kernel; returns (full_output, exec_time_ns or None)."""
    nc = _get_nc()
    in_maps = _shard_inputs(**inputs)
    res = run_bass_kernel_spmd(nc, in_maps, core_ids=list(range(8)), trace=trace)
    out = np.empty((B, S, D), np.float32)
    for b in range(B):
        acc = res.results[2 * b]["outT"] + res.results[2 * b + 1]["outT"]
        out[b] = acc.T
    return out, res.exec_time_ns


def kernel(x, w_qkv, b_qkv, w_out, b_out):
    out, _ = run(dict(x=x, w_qkv=w_qkv, b_qkv=b_qkv, w_out=w_out, b_out=b_out))
    return out


# revision 15
# speedup vs baseline: 1.1907x; 1.1907x over previous
"""Multi-head self-attention (B=4, S=2048, D=1024, H=16) on 8 TRN2 NeuronCores.

Sharding: core i = (batch b = i//2, head-group g = i%2): 8 heads per core,
Megatron-style partial output projection summed on host.

v2 (bf16): all matmuls in bf16 (PE runs bf16 at 1 col/cycle with ~0
per-instruction overhead vs ~90 cycles for f32r). Scores use the PE
quadrant-tiling: head 2f lives on partitions 0-63 of qt/kt f-tiles, head
2f+1 on 64-127; the two K=64 score matmuls of a head pair execute in
disjoint quadrant rows and overlap on HW (measured 108ns each for N=512,
i.e. 2x). Softmax denominators come free from a ones-column in V_aug
(M=65 PV matmuls cost the same as M=128). exp runs on ACT (the critical
path, ~277us); emission order starts it as early as possible (K proj ->
V proj -> per-s-chunk Q proj + attention) and keeps it streaming.
"""
import os
import sys
import types

import numpy as np

# ---------------------------------------------------------------------------
# environment bootstrap (self-contained: no problem-dir imports)
# ---------------------------------------------------------------------------


def _install_ntff_hook():
    """run_bass_kernel_spmd(trace=True) under axon needs antenv.axon_hooks,
    which the agent image's antenv stub lacks. Recreate it."""
    if "antenv.axon_hooks" in sys.modules:
        return
    try:
        import antenv
        from trn_agent_boot.trn_boot import _ntff_profile_via_ctypes
    except Exception:
        return
    so_path = "/opt/axon/libaxon_pjrt.so"
    if not os.path.exists(so_path):
        return
    mod = types.ModuleType("antenv.axon_hooks")
    _hook = [_ntff_profile_via_ctypes(so_path)]
    mod.get_axon_ntff_profile_hook = lambda: _hook[0]

    def _set(h):
        _hook[0] = h

    mod.set_axon_ntff_profile_hook = _set
    sys.modules["antenv.axon_hooks"] = mod
    antenv.axon_hooks = mod


_install_ntff_hook()

import concourse.bacc as bacc
import concourse.tile as tile
from concourse import mybir
from concourse.bass_utils import run_bass_kernel_spmd
from contextlib import ExitStack

# ---------------------------------------------------------------------------
# problem constants (hardcoded per contract)
# ---------------------------------------------------------------------------
B, S, D = 4, 2048, 1024
H, HD = 16, 64
HPG = 8            # heads per core (group)
E = HPG * HD       # 512 attention features per core
P = 128
SC = 512           # s-chunk
NS = S // SC       # 4 s-chunks
NT = S // P        # 16 t-chunks
ND = D // P        # 8 d-chunks
NF = E // P        # 4 f-tiles (head pairs)
HD1 = HD + 1       # V_aug columns per head (V + ones)
SCALE = 1.0 / np.sqrt(np.float32(HD))

F32 = mybir.dt.float32
BF16 = mybir.dt.bfloat16
EXP = mybir.ActivationFunctionType.Exp

_NC_CACHE = {}


def _build_nc():
    nc = bacc.Bacc("TRN2", target_bir_lowering=False)

    xT = nc.dram_tensor("xT", [D, S], BF16, kind="ExternalInput")
    wqT = nc.dram_tensor("wqT", [D, E], BF16, kind="ExternalInput")
    wkT = nc.dram_tensor("wkT", [D, E], BF16, kind="ExternalInput")
    wvT = nc.dram_tensor("wvT", [D, E], BF16, kind="ExternalInput")
    woT = nc.dram_tensor("woT", [E, D], BF16, kind="ExternalInput")
    bq = nc.dram_tensor("bq", [E, 1], F32, kind="ExternalInput")
    bk = nc.dram_tensor("bk", [E, 1], F32, kind="ExternalInput")
    bv = nc.dram_tensor("bv", [1, E], F32, kind="ExternalInput")
    bo = nc.dram_tensor("bo", [D, 1], F32, kind="ExternalInput")
    outT = nc.dram_tensor("outT", [D, S], F32, kind="ExternalOutput")

    with tile.TileContext(nc) as tc, ExitStack() as glob:
        const = glob.enter_context(tc.tile_pool(name="const", bufs=1))
        # resident inputs
        xr = [const.tile([P, S], BF16, name=f"xr{d}") for d in range(ND)]
        wq = [const.tile([P, E], BF16, name=f"wq{d}") for d in range(ND)]
        wk = [const.tile([P, E], BF16, name=f"wk{d}") for d in range(ND)]
        wv = [const.tile([P, E], BF16, name=f"wv{d}") for d in range(ND)]
        wo = [const.tile([P, D], BF16, name=f"wo{e}") for e in range(NF)]
        # DMA order: K-proj inputs first (they gate the first scores)
        for d in range(ND):
            nc.sync.dma_start(xr[d][:], xT[d * P:(d + 1) * P, :])
            nc.sync.dma_start(wk[d][:], wkT[d * P:(d + 1) * P, :])
        for d in range(ND):
            nc.sync.dma_start(wq[d][:], wqT[d * P:(d + 1) * P, :])
        for d in range(ND):
            nc.sync.dma_start(wv[d][:], wvT[d * P:(d + 1) * P, :])
        for e in range(NF):
            nc.sync.dma_start(wo[e][:], woT[e * P:(e + 1) * P, :])
        bqt = [const.tile([P, 1], F32, name=f"bqt{f}") for f in range(NF)]
        bkt = [const.tile([P, 1], F32, name=f"bkt{f}") for f in range(NF)]
        for f in range(NF):
            nc.sync.dma_start(bqt[f][:], bq[f * P:(f + 1) * P, :])
            nc.sync.dma_start(bkt[f][:], bk[f * P:(f + 1) * P, :])
        bv_bc = const.tile([P, E], F32, name="bv_bc")
        nc.sync.dma_start(bv_bc[:], bv[0:1, :].to_broadcast((P, E)))
        bot = [const.tile([P, 1], F32, name=f"bot{i}") for i in range(ND)]
        for i in range(ND):
            nc.sync.dma_start(bot[i][:], bo[i * P:(i + 1) * P, :])

        resid = glob.enter_context(tc.tile_pool(name="resid", bufs=1))
        qt = [resid.tile([P, S], BF16, name=f"qt{f}") for f in range(NF)]
        kt = [resid.tile([P, S], BF16, name=f"kt{f}") for f in range(NF)]
        vt = [resid.tile([P, HPG * HD1], BF16, name=f"vt{t}") for t in range(NT)]

        # psum budget (8 banks): sc ring 2x[128,1024]=4, o pool 2x[65,512]=2
        # (bufs=1; eviction slack comes from the delayed PV chain start),
        # proj ring 2x[128,512]=2 for all projection/out-proj chains.
        R = glob.enter_context(tc.tile_pool(name="R", bufs=2, space="PSUM"))
        PJ = glob.enter_context(tc.tile_pool(name="PJ", bufs=2, space="PSUM"))
        opool = glob.enter_context(tc.tile_pool(name="opool", bufs=1, space="PSUM"))

        pt_pool = glob.enter_context(tc.tile_pool(name="pt", bufs=16))
        oc_pool = glob.enter_context(tc.tile_pool(name="oc", bufs=2))
        nrm_pool = glob.enter_context(tc.tile_pool(name="nrm", bufs=2))
        on_pool = glob.enter_context(tc.tile_pool(name="on", bufs=2))
        ot_pool = glob.enter_context(tc.tile_pool(name="ot", bufs=3))
        dram_pool = glob.enter_context(tc.tile_pool(name="dramrs", bufs=2, space="DRAM"))

        # ------------------------------------------------------------------
        # Emission engine: ACT (exp) is the critical path at ~1.08us per
        # [128,1024] tile; the PE must stream score tiles at that cadence
        # while folding ALL projection work into the leftover slots so it
        # never idles long (PE DVFS: stalls drop the clock to 1.2GHz).
        # ------------------------------------------------------------------

        class FillerStream:
            """Pops projection matmuls a few at a time, managing chain state."""

            def __init__(self):
                self.chains = []   # list of (mm_list, fin_fn)
                self.cur = None
                self.idx = 0

            def push_chain(self, mms, fin):
                self.chains.append((mms, fin))

            def pop(self, n):
                done = 0
                while done < n:
                    if self.cur is None:
                        if not self.chains:
                            return done
                        self.cur = self.chains.pop(0)
                        self.idx = 0
                    mms, fin = self.cur
                    mms[self.idx]()
                    self.idx += 1
                    done += 1
                    if self.idx == len(mms):
                        fin()
                        self.cur = None
                return done

        fill = FillerStream()

        def kproj_chain(f, s):
            sl = slice(s * SC, (s + 1) * SC)
            box = {}

            def mk(d):
                def mm():
                    if d == 0:
                        box["t"] = PJ.tile([P, SC], F32, name="kps", tag="PJ")
                    nc.tensor.matmul(
                        box["t"][:], wk[d][:, f * P:(f + 1) * P], xr[d][:, sl],
                        start=(d == 0), stop=(d == ND - 1))
                return mm

            def fin():
                nc.vector.tensor_scalar_add(kt[f][:, sl], box["t"][:], bkt[f][:])

            return [mk(d) for d in range(ND)], fin

        def qproj_chain(f, s):
            sl = slice(s * SC, (s + 1) * SC)
            box = {}

            def mk(d):
                def mm():
                    if d == 0:
                        box["t"] = PJ.tile([P, SC], F32, name="qps", tag="PJ")
                    nc.tensor.matmul(
                        box["t"][:], wq[d][:, f * P:(f + 1) * P], xr[d][:, sl],
                        start=(d == 0), stop=(d == ND - 1))
                return mm

            def fin():
                nc.vector.tensor_scalar_add(qt[f][:, sl], box["t"][:], bqt[f][:])

            return [mk(d) for d in range(ND)], fin

        def vproj_chain(t):
            d0 = t * P
            box = {}

            def mk(d):
                def mm():
                    if d == 0:
                        box["t"] = PJ.tile([P, SC], F32, name="vps", tag="PJ")
                    nc.tensor.matmul(
                        box["t"][:], xr[d][:, d0:d0 + P], wv[d][:],
                        start=(d == 0), stop=(d == ND - 1))
                return mm

            def fin():
                vdst = vt[t][:].rearrange("p (h c) -> p h c", c=HD1)
                nc.vector.tensor_add(
                    vdst[:, :, 0:HD],
                    box["t"][:].rearrange("p (h c) -> p h c", c=HD),
                    bv_bc[:].rearrange("p (h c) -> p h c", c=HD))
                nc.vector.memset(vdst[:, :, HD:HD1], 1.0)

            return [mk(d) for d in range(ND)], fin

        def outproj_chain(dc, s, on_tiles):
            sl = slice(s * SC, (s + 1) * SC)
            box = {}

            def mk(e):
                def mm():
                    if e == 0:
                        box["t"] = PJ.tile([P, SC], F32, name="opps", tag="PJ")
                    nc.tensor.matmul(
                        box["t"][:], wo[e][:, dc * P:(dc + 1) * P], on_tiles[e][:],
                        start=(e == 0), stop=(e == NF - 1))
                return mm

            def fin():
                ot = ot_pool.tile([P, SC], F32, name="ottile", tag="ot")
                nc.vector.tensor_scalar_add(ot[:], box["t"][:], bot[dc][:])
                nc.sync.dma_start(outT[dc * P:(dc + 1) * P, sl], ot[:])

            return [mk(e) for e in range(NF)], fin

        # score + exp for global step g
        pt_tiles = {}

        def sc_exp(g):
            s, hp, t = g // 64, (g // 16) % 4, g % 16
            sl = slice(s * SC, (s + 1) * SC)
            tsl = slice(t * P, (t + 1) * P)
            sc_ps = R.tile([P, 2 * SC], F32, name="scps", tag="R")
            nc.tensor.matmul(
                sc_ps[:, 0:SC], kt[hp][0:HD, tsl], qt[hp][0:HD, sl],
                start=True, stop=True, tile_position=(0, 0))
            nc.tensor.matmul(
                sc_ps[:, SC:2 * SC], kt[hp][HD:P, tsl], qt[hp][HD:P, sl],
                start=True, stop=True, tile_position=(HD, 0))
            pt = pt_pool.tile([P, 2 * SC], BF16, name="ptile", tag="pt")
            nc.scalar.activation(pt[:], sc_ps[:], EXP, scale=float(SCALE))
            pt_tiles[g] = pt

        # PV pair for global step g' (consumes pt_tiles[g'])
        ostate = {}   # block -> (o_psA, o_psB)
        on_all = {}   # s -> [on tiles]

        def pv_pair(gp):
            blk = gp // 16
            s, hp, t = gp // 64, (gp // 16) % 4, gp % 16
            hA, hB = 2 * hp, 2 * hp + 1
            if t == 0:
                ostate[blk] = (
                    opool.tile([HD1, SC], F32, name="opsA", tag="oA"),
                    opool.tile([HD1, SC], F32, name="opsB", tag="oB"))
            o_psA, o_psB = ostate[blk]
            pt = pt_tiles.pop(gp)
            nc.tensor.matmul(
                o_psA[:], vt[t][:, hA * HD1:(hA + 1) * HD1], pt[:, 0:SC],
                start=(t == 0), stop=(t == NT - 1))
            nc.tensor.matmul(
                o_psB[:], vt[t][:, hB * HD1:(hB + 1) * HD1], pt[:, SC:2 * SC],
                start=(t == 0), stop=(t == NT - 1))
            if t == NT - 1:
                norm_block(s, hp, o_psA, o_psB)

        def norm_block(s, hp, o_psA, o_psB):
            """Evict O_aug, normalize by the ones-row sums into on_all[s][hp]."""
            ocA = oc_pool.tile([HD1, SC], F32, name="ocA", tag="ocA")
            ocB = oc_pool.tile([HD1, SC], F32, name="ocB", tag="ocB")
            nc.vector.tensor_copy(ocA[:], o_psA[:])
            nc.vector.tensor_copy(ocB[:], o_psB[:])
            rcpf = nrm_pool.tile([2, SC], F32, name="rcpf", tag="rcpf")
            nc.sync.dma_start(rcpf[0:1, :], ocA[HD:HD1, :])
            nc.sync.dma_start(rcpf[1:2, :], ocB[HD:HD1, :])
            rcpv = nrm_pool.tile([2, SC], F32, name="rcpv", tag="rcpv")
            nc.vector.reciprocal_approx_fast(rcpv[:], rcpf[:])
            rd = dram_pool.tile([2, SC], F32, name="rdtile", tag="rd")
            nc.sync.dma_start(rd[:, :], rcpv[:])
            rbA = nrm_pool.tile([HD, SC], F32, name="rbA", tag="rbA")
            rbB = nrm_pool.tile([HD, SC], F32, name="rbB", tag="rbB")
            nc.sync.dma_start(rbA[:], rd[0:1, :].to_broadcast((HD, SC)))
            nc.sync.dma_start(rbB[:], rd[1:2, :].to_broadcast((HD, SC)))
            on = on_all[s][hp]
            nc.vector.tensor_mul(on[0:HD, :], ocA[0:HD, :], rbA[:])
            tmpB = nrm_pool.tile([HD, SC], BF16, name="tmpB", tag="tmpB")
            nc.vector.tensor_mul(tmpB[:], ocB[0:HD, :], rbB[:])
            nc.sync.dma_start(on[HD:P, :], tmpB[:])
            if hp == NF - 1:
                for dc in range(ND):
                    fill.push_chain(*outproj_chain(dc, s, on_all[s]))

        # ---------------- prefix ------------------------------------------
        on_all[0] = [on_pool.tile([P, SC], BF16, name="on", tag=f"on{hp}")
                     for hp in range(NF)]
        # K f0 (all four s-chunk chains), Q s0 f0 -- inline, back to back
        for s in range(NS):
            mms, fin = kproj_chain(0, s)
            for m in mms:
                m()
            fin()
        mms, fin = qproj_chain(0, 0)
        for m in mms:
            m()
        fin()
        # V projection interleaved with hp0's scores (feeds ACT early)
        for t in range(NT):
            sc_exp(t)
            mms, fin = vproj_chain(t)
            for m in mms:
                m()
            fin()
        # K f1 + Q s0 f1 before the steady loop needs them
        for s in range(NS):
            mms, fin = kproj_chain(1, s)
            for m in mms:
                m()
            fin()
        mms, fin = qproj_chain(1, 0)
        for m in mms:
            m()
        fin()

        # ---------------- steady loop -------------------------------------
        DELAY = 4      # PV pointer trails the sc pointer by >= DELAY steps
        GAP = 3        # steps between a chain's stop and the next chain start
        pv_ptr = 0
        last_stop = [-10]

        def chase(g, maxpairs):
            nonlocal pv_ptr
            emitted = 0
            while emitted < maxpairs and pv_ptr <= g - DELAY and pv_ptr < 256:
                if pv_ptr % 16 == 0 and g - last_stop[0] < GAP:
                    break
                pv_pair(pv_ptr)
                if pv_ptr % 16 == 15:
                    last_stop[0] = g
                pv_ptr += 1
                emitted += 1
            return emitted

        for g in range(NT, 4 * 64):
            s, hp, t = g // 64, (g // 16) % 4, g % 16
            if t == 0:
                if hp == 0:
                    on_all[s] = [on_pool.tile([P, SC], BF16, name="on",
                                              tag=f"on{q}") for q in range(NF)]
                    if 0 < s < NS - 1:
                        for f in range(NF):
                            fill.push_chain(*qproj_chain(f, s + 1))
                # s0: K/Q for the next head pair, Q chain first (tightest
                # deadline: next block's first scores read it immediately)
                if s == 0 and 1 <= hp < NF - 1:
                    fill.push_chain(*qproj_chain(hp + 1, 0))
                    for sk in range(NS):
                        fill.push_chain(*kproj_chain(hp + 1, sk))
                if s == 0 and hp == NF - 1:
                    for f in range(NF):
                        fill.push_chain(*qproj_chain(f, 1))
            # fillers FIRST: their writes must precede this step's score
            # matmuls in program order (the tile framework only orders
            # writes->reads that appear in emission order)
            fill.pop(3 if s == 0 else 1)
            sc_exp(g)
            npairs = chase(g, 3)
            if s > 0 and npairs < 2:
                fill.pop(2 - npairs)
        # drain: remaining PV pairs, then leftover fillers (incl. outproj s3)
        g = 4 * 64
        while pv_ptr < 256:
            chase(g, 4)
            fill.pop(2)
            g += 1
        while fill.pop(8):
            pass

    nc.finalize()
    return nc


def _get_nc():
    if "nc" not in _NC_CACHE:
        _NC_CACHE["nc"] = _build_nc()
    return _NC_CACHE["nc"]


def _shard_inputs(x, w_qkv, b_qkv, w_out, b_out):
    """Build the 8 per-core input maps. Core i = (b = i//2, g = i%2)."""
    import ml_dtypes
    bf16 = ml_dtypes.bfloat16
    x = np.asarray(x, np.float32)
    w_qkv = np.asarray(w_qkv, np.float32)
    b_qkv = np.asarray(b_qkv, np.float32)
    w_out = np.asarray(w_out, np.float32)
    b_out = np.asarray(b_out, np.float32)

    in_maps = []
    for b in range(B):
        xTb = np.ascontiguousarray(x[b].T.astype(bf16))  # [D, S]
        for g in range(2):
            heads = range(g * HPG, (g + 1) * HPG)
            # w_qkv rows for head h: [192h, 192h+64) = Q, +64..128 = K, +128..192 = V
            q_rows = np.concatenate(
                [np.arange(3 * HD * h, 3 * HD * h + HD) for h in heads])
            k_rows = q_rows + HD
            v_rows = q_rows + 2 * HD
            wqT = np.ascontiguousarray(w_qkv[q_rows].T.astype(bf16))  # [D, E]
            wkT = np.ascontiguousarray(w_qkv[k_rows].T.astype(bf16))
            wvT = np.ascontiguousarray(w_qkv[v_rows].T.astype(bf16))
            ecols = np.arange(g * E, (g + 1) * E)
            woT = np.ascontiguousarray(w_out[:, ecols].T.astype(bf16))  # [E, D]
            bo_ = b_out[:, None] if g == 0 else np.zeros((D, 1), np.float32)
            in_maps.append({
                "xT": xTb,
                "wqT": wqT,
                "wkT": wkT,
                "wvT": wvT,
                "woT": woT,
                "bq": np.ascontiguousarray(b_qkv[q_rows][:, None]),
                "bk": np.ascontiguousarray(b_qkv[k_rows][:, None]),
                "bv": np.ascontiguousarray(b_qkv[v_rows][None, :]),
                "bo": np.ascontiguousarray(bo_),
            })
    return in_maps


def run(inputs, trace=False):
    """Run the kernel; returns (full_output, exec_time_ns or None)."""
    nc = _get_nc()
    in_maps = _shard_inputs(**inputs)
    res = run_bass_kernel_spmd(nc, in_maps, core_ids=list(range(8)), trace=trace)
    out = np.empty((B, S, D), np.float32)
    for b in range(B):
        acc = res.results[2 * b]["outT"] + res.results[2 * b + 1]["outT"]
        out[b] = acc.T
    return out, res.exec_time_ns


def kernel(x, w_qkv, b_qkv, w_out, b_out):
    out, _ = run(dict(x=x, w_qkv=w_qkv, b_qkv=b_qkv, w_out=w_out, b_out=b_out))
    return out
